# revision 7
# baseline (speedup 1.0000x reference)
"""LoRA-attention TRN2 kernel: head-tensor-parallel over 8 NeuronCores.

Problem (hardcoded): x [4, 2048, 2048] f32, causal mask, H=16 heads, HD=128,
LoRA rank 16 on all four projections.

Strategy:
  - Host folds LoRA into the weights:  W_eff^T = W^T + A^T @ B^T  (exact).
  - Tensor-parallel over heads: core c owns heads {2c, 2c+1} = feature slice
    [c*256, (c+1)*256).  Each core computes Q^T/K^T (feature-major) and V
    (token-major) for its slice, causal flash-style attention per (batch,
    head) with logits kept transposed ([key, query]) so the attention
    contraction stays on the partition dim, then a partial output projection
    over its 256 features.  Host sums the 8 partial outputs.
  - Softmax: no max-subtraction (logits are O(1); exp of masked -inf never
    occurs because masked tiles are either skipped or zeroed post-exp).
    Column sums via ones-vector matmul on the PE; normalization applied to
    O^T at the end via a PE broadcast (ones outer product) + DVE multiply.
"""

import math
import os
import sys

import numpy as np

sys.path.insert(0, "/opt/trn_rl_repo")

B, S, D, H, R = 4, 2048, 2048, 16, 16
HD = D // H              # 128
NCORES = 8
HPC = H // NCORES        # heads per core = 2
FPC = HPC * HD           # features per core = 256
T = B * S                # 8192 tokens
P = 128
SCALE = 1.0 / math.sqrt(HD)

_COMPILED = {}


def _build_nc(causal: bool, iters: int = 1):
    import concourse.mybir as mybir
    import concourse.tile as tile
    from concourse import bacc

    f32 = mybir.dt.float32
    nc = bacc.Bacc("TRN2", target_bir_lowering=False, debug=False)

    xt = nc.dram_tensor("xt", [D, T], f32, kind="ExternalInput")
    wq = nc.dram_tensor("wq", [D, FPC], f32, kind="ExternalInput")
    wk = nc.dram_tensor("wk", [D, FPC], f32, kind="ExternalInput")
    wv = nc.dram_tensor("wv", [D, FPC], f32, kind="ExternalInput")
    wo = nc.dram_tensor("wo", [FPC, D], f32, kind="ExternalInput")
    dm = nc.dram_tensor("dm", [4 * P, 512], f32, kind="ExternalInput")
    out = nc.dram_tensor("out", [T, D], f32, kind="ExternalOutput")

    KC = D // P            # 16 contraction chunks for projections
    NTG = S // 512         # 4 token groups per batch
    NIT = S // 512         # 4 query tiles per (b, h)
    NJT = S // P           # 16 key chunks per (b, h)

    with tile.TileContext(nc) as tc:
        with (
            tc.tile_pool(name="consts", bufs=1) as consts,
            tc.tile_pool(name="xp", bufs=4) as xp,
            tc.tile_pool(name="qk", bufs=1) as qkp,
            tc.tile_pool(name="vp", bufs=1) as vpool,
            tc.tile_pool(name="ep", bufs=4) as ep,
            tc.tile_pool(name="ot", bufs=1) as otp_pool,
            tc.tile_pool(name="sm", bufs=4) as smp,
            tc.tile_pool(name="ob", bufs=4) as obp,
            tc.tile_pool(name="ps", bufs=8, space="PSUM") as ps,
        ):
            # ---- resident constants ----
            wq_sb = consts.tile([P, KC, FPC], f32)
            wk_sb = consts.tile([P, KC, FPC], f32)
            wv_sb = consts.tile([P, KC, FPC], f32)
            wo_sb = consts.tile([P, HPC, D], f32)
            nc.sync.dma_start(out=wq_sb, in_=wq.ap().rearrange("(c p) f -> p c f", p=P))
            nc.sync.dma_start(out=wk_sb, in_=wk.ap().rearrange("(c p) f -> p c f", p=P))
            nc.sync.dma_start(out=wv_sb, in_=wv.ap().rearrange("(c p) f -> p c f", p=P))
            nc.sync.dma_start(out=wo_sb, in_=wo.ap().rearrange("(h p) e -> p h e", p=P))
            dm_sb = consts.tile([P, 4, 512], f32)
            nc.sync.dma_start(out=dm_sb, in_=dm.ap().rearrange("(d p) i -> p d i", p=P))
            ones_col = consts.tile([P, 1], f32)   # K=128, M=1 sums weight
            nc.any.memset(ones_col, 1.0)
            ones_row = consts.tile([1, P], f32)   # K=1, M=128 broadcast weight
            nc.any.memset(ones_row, 1.0)

            def body():
                for b in range(B):
                    _batch(b)

            def _batch(b):
                # ---------- QKV projections for batch b ----------
                qt_sb = qkp.tile([P, HPC, S], f32, tag="qt")
                kt_sb = qkp.tile([P, HPC, S], f32, tag="kt")
                v_sb = vpool.tile([P, NJT, FPC], f32, tag="v")
                for g in range(NTG):
                    toff = b * S + g * 512
                    qps = [ps.tile([P, 512], f32, tag="ps", name=f"qp{b}_{g}_{i}")
                           for i in range(HPC)]
                    kps = [ps.tile([P, 512], f32, tag="ps", name=f"kp{b}_{g}_{i}")
                           for i in range(HPC)]
                    vps = [ps.tile([P, FPC], f32, tag="ps", name=f"vp{b}_{g}_{i}")
                           for i in range(4)]
                    for kc in range(KC):
                        xt_t = xp.tile([P, 512], f32, tag="xt")
                        nc.sync.dma_start(
                            out=xt_t, in_=xt.ap()[kc * P:(kc + 1) * P, toff:toff + 512]
                        )
                        st = dict(start=(kc == 0), stop=(kc == KC - 1))
                        for m in range(HPC):
                            nc.tensor.matmul(
                                qps[m], wq_sb[:, kc, m * P:(m + 1) * P], xt_t, **st
                            )
                            nc.tensor.matmul(
                                kps[m], wk_sb[:, kc, m * P:(m + 1) * P], xt_t, **st
                            )
                        for ts in range(4):
                            nc.tensor.matmul(
                                vps[ts], xt_t[:, ts * P:(ts + 1) * P],
                                wv_sb[:, kc, :], **st
                            )
                    for m in range(HPC):
                        nc.vector.tensor_copy(
                            qt_sb[:, m, g * 512:(g + 1) * 512], qps[m]
                        )
                        nc.vector.tensor_copy(
                            kt_sb[:, m, g * 512:(g + 1) * 512], kps[m]
                        )
                    for ts in range(4):
                        nc.vector.tensor_copy(v_sb[:, g * 4 + ts, :], vps[ts])

                # ---------- attention per head ----------
                ot_sb = otp_pool.tile([P, HPC, S], f32, tag="ot")
                for h in range(HPC):
                    for it in range(NIT):
                        otp = ps.tile([P, 512], f32, tag="ps")
                        sp = ps.tile([1, 512], f32, tag="ps")
                        jts = range(4 * it + 4) if causal else range(NJT)
                        jlast = (4 * it + 3) if causal else (NJT - 1)
                        for jt in jts:
                            lp = ps.tile([P, 512], f32, tag="ps")
                            nc.tensor.matmul(
                                lp, kt_sb[:, h, jt * P:(jt + 1) * P],
                                qt_sb[:, h, it * 512:(it + 1) * 512],
                                start=True, stop=True,
                            )
                            e = ep.tile([P, 512], f32, tag="e")
                            nc.scalar.activation(
                                e, lp, mybir.ActivationFunctionType.Exp,
                                scale=SCALE,
                            )
                            if causal and jt >= 4 * it:
                                nc.vector.tensor_mul(e, e, dm_sb[:, jt - 4 * it, :])
                            stj = dict(start=(jt == 0), stop=(jt == jlast))
                            nc.tensor.matmul(
                                otp, v_sb[:, jt, h * P:(h + 1) * P], e, **stj
                            )
                            nc.tensor.matmul(sp, ones_col, e, **stj)
                        rinv = smp.tile([1, 512], f32, tag="rinv")
                        nc.vector.reciprocal(rinv, sp)
                        rbc = ps.tile([P, 512], f32, tag="ps")
                        nc.tensor.matmul(rbc, ones_row, rinv, start=True, stop=True)
                        rbs = smp.tile([P, 512], f32, tag="rbs")
                        nc.vector.tensor_copy(rbs, rbc)
                        nc.vector.tensor_mul(
                            ot_sb[:, h, it * 512:(it + 1) * 512], otp, rbs
                        )

                # ---------- partial output projection for batch b ----------
                for mt in range(S // P):
                    for nt in range(D // 512):
                        op = ps.tile([P, 512], f32, tag="ps")
                        for h in range(HPC):
                            nc.tensor.matmul(
                                op, ot_sb[:, h, mt * P:(mt + 1) * P],
                                wo_sb[:, h, nt * 512:(nt + 1) * 512],
                                start=(h == 0), stop=(h == HPC - 1),
                            )
                        ob = obp.tile([P, 512], f32, tag="ob")
                        nc.vector.tensor_copy(ob, op)
                        nc.sync.dma_start(
                            out=out.ap()[
                                b * S + mt * P: b * S + (mt + 1) * P,
                                nt * 512:(nt + 1) * 512,
                            ],
                            in_=ob,
                        )

            if iters > 1:
                with tc.For_i(0, iters, 1):
                    body()
            else:
                body()
    nc.compile()
    return nc


def _get_nc(causal: bool):
    if causal not in _COMPILED:
        _COMPILED[causal] = _build_nc(causal)
    return _COMPILED[causal]


def _numpy_reference(x, mask, Wq, Aq, Bq, Wk, Ak, Bk, Wv, Av, Bv, Wo, Ao, Bo):
    def lora(x2, W, A, Bm):
        return x2 @ W.T + (x2 @ A.T) @ Bm.T

    b, s, d = x.shape
    x2 = x.reshape(b * s, d)

    def heads(t):
        return t.reshape(b, s, H, HD).transpose(0, 2, 1, 3)

    Q = heads(lora(x2, Wq, Aq, Bq).reshape(b, s, d))
    K = heads(lora(x2, Wk, Ak, Bk).reshape(b, s, d))
    V = heads(lora(x2, Wv, Av, Bv).reshape(b, s, d))
    attn = np.einsum("bhqd,bhkd->bhqk", Q, K) / math.sqrt(HD)
    attn = np.where(mask == 0, np.float32(-1e9), attn)
    attn = attn - attn.max(axis=-1, keepdims=True)
    attn = np.exp(attn)
    attn = attn / attn.sum(axis=-1, keepdims=True)
    o = np.einsum("bhqk,bhkd->bhqd", attn, V)
    o = o.transpose(0, 2, 1, 3).reshape(b * s, d)
    return lora(o, Wo, Ao, Bo).reshape(b, s, d).astype(np.float32)


def kernel(x, mask, Wq, Aq, Bq, Wk, Ak, Bk, Wv, Av, Bv, Wo, Ao, Bo):
    from concourse.bass_utils import run_bass_kernel_spmd

    x = np.asarray(x, dtype=np.float32)
    m2 = np.asarray(mask).reshape(S, S)
    if np.array_equal(m2, np.tril(np.ones((S, S), m2.dtype))):
        causal = True
    elif np.all(m2 != 0):
        causal = False
    else:
        return _numpy_reference(
            np.asarray(x), np.asarray(mask),
            *(np.asarray(a) for a in (Wq, Aq, Bq, Wk, Ak, Bk, Wv, Av, Bv, Wo, Ao, Bo)),
        )

    # Fold LoRA into effective (transposed) weights: W_eff^T = W^T + A^T B^T.
    wqt = (np.asarray(Wq).T + np.asarray(Aq).T @ np.asarray(Bq).T).astype(np.float32)
    wkt = (np.asarray(Wk).T + np.asarray(Ak).T @ np.asarray(Bk).T).astype(np.float32)
    wvt = (np.asarray(Wv).T + np.asarray(Av).T @ np.asarray(Bv).T).astype(np.float32)
    wot = (np.asarray(Wo).T + np.asarray(Ao).T @ np.asarray(Bo).T).astype(np.float32)

    xt = np.ascontiguousarray(x.reshape(T, D).T)

    # Diagonal-crossing causal masks: tile (jt=4*it+d, it): keep where
    # 128*d + p_j <= f_i.
    dmn = np.zeros((4, P, 512), dtype=np.float32)
    for dd in range(4):
        pj = np.arange(P)[:, None]
        fi = np.arange(512)[None, :]
        dmn[dd] = (P * dd + pj <= fi).astype(np.float32)
    dmn = dmn.reshape(4 * P, 512)

    nc = _get_nc(causal)
    in_maps = []
    for c in range(NCORES):
        fs = slice(c * FPC, (c + 1) * FPC)
        in_maps.append({
            "xt": xt,
            "wq": np.ascontiguousarray(wqt[:, fs]),
            "wk": np.ascontiguousarray(wkt[:, fs]),
            "wv": np.ascontiguousarray(wvt[:, fs]),
            "wo": np.ascontiguousarray(wot[fs, :]),
            "dm": dmn,
        })
    res = run_bass_kernel_spmd(
        nc, in_maps, list(range(NCORES)),
        trace=bool(int(os.environ.get("KERNEL_TRACE", "0"))),
    )
    if os.environ.get("KERNEL_TRACE") and res.exec_time_ns is not None:
        print(f"HW exec time: {res.exec_time_ns} ns", file=sys.stderr)
    total = np.zeros((T, D), dtype=np.float32)
    for r in res.results:
        total += r["out"]
    return total.reshape(B, S, D)


# revision 11
# speedup vs baseline: 2.8475x; 2.8475x over previous
"""LoRA-attention TRN2 kernel: head-tensor-parallel over 8 NeuronCores.

Problem (hardcoded): x [4, 2048, 2048] f32, causal mask, H=16 heads, HD=128,
LoRA rank 16 on all four projections.

Strategy:
  - Host folds LoRA into the weights:  W_eff^T = W^T + A^T @ B^T  (exact).
  - Tensor-parallel over heads: core c owns heads {2c, 2c+1} = feature slice
    [c*256, (c+1)*256).  Each core computes Q^T/K^T (feature-major) and V
    (token-major) for its slice, causal flash-style attention per (batch,
    head) with logits kept transposed ([key, query]) so the attention
    contraction stays on the partition dim, then a partial output projection
    over its 256 features.  Host sums the 8 partial outputs.
  - Softmax: no max-subtraction (logits are O(1); exp of masked -inf never
    occurs because masked tiles are either skipped or zeroed post-exp).
    Column sums via ones-vector matmul on the PE; normalization applied to
    O^T at the end via a PE broadcast (ones outer product) + DVE multiply.
"""

import math
import os
import sys

import numpy as np

sys.path.insert(0, "/opt/trn_rl_repo")

B, S, D, H, R = 4, 2048, 2048, 16, 16
HD = D // H              # 128
NCORES = 8
HPC = H // NCORES        # heads per core = 2
FPC = HPC * HD           # features per core = 256
T = B * S                # 8192 tokens
P = 128
SCALE = 1.0 / math.sqrt(HD)

_COMPILED = {}


def _build_nc(causal: bool, iters: int = 1):
    import concourse.mybir as mybir
    import concourse.tile as tile
    from concourse import bacc

    f32 = mybir.dt.float32
    f32r = mybir.dt.float32r
    nc = bacc.Bacc("TRN2", target_bir_lowering=False, debug=False)

    xt = nc.dram_tensor("xt", [D, T], f32r, kind="ExternalInput")
    wq = nc.dram_tensor("wq", [D, FPC], f32r, kind="ExternalInput")
    wk = nc.dram_tensor("wk", [D, FPC], f32r, kind="ExternalInput")
    wv = nc.dram_tensor("wv", [D, FPC], f32r, kind="ExternalInput")
    wo = nc.dram_tensor("wo", [FPC, D], f32r, kind="ExternalInput")
    dm = nc.dram_tensor("dm", [4 * P, 512], f32, kind="ExternalInput")
    out = nc.dram_tensor("out", [T, D], f32, kind="ExternalOutput")

    KC = D // P            # 16 contraction chunks for projections
    NTG = S // 512         # 4 token groups per batch
    NIT = S // 512         # 4 query tiles per (b, h)
    NJT = S // P           # 16 key chunks per (b, h)

    with tile.TileContext(nc) as tc:
        with (
            nc.allow_low_precision(reason="f32r matmul pipeline; fp32 PSUM accum"),
            tc.tile_pool(name="consts", bufs=1) as consts,
            tc.tile_pool(name="xp", bufs=4) as xp,
            tc.tile_pool(name="qk", bufs=1) as qkp,
            tc.tile_pool(name="vp", bufs=1) as vpool,
            tc.tile_pool(name="ep", bufs=4) as ep,
            tc.tile_pool(name="ot", bufs=1) as otp_pool,
            tc.tile_pool(name="sm", bufs=4) as smp,
            tc.tile_pool(name="ob", bufs=4) as obp,
            tc.tile_pool(name="ps", bufs=8, space="PSUM") as ps,
        ):
            # ---- resident constants ----
            wq_sb = consts.tile([P, KC, FPC], f32r)
            wk_sb = consts.tile([P, KC, FPC], f32r)
            wv_sb = consts.tile([P, KC, FPC], f32r)
            wo_sb = consts.tile([P, HPC, D], f32r)
            nc.sync.dma_start(out=wq_sb, in_=wq.ap().rearrange("(c p) f -> p c f", p=P))
            nc.sync.dma_start(out=wk_sb, in_=wk.ap().rearrange("(c p) f -> p c f", p=P))
            nc.sync.dma_start(out=wv_sb, in_=wv.ap().rearrange("(c p) f -> p c f", p=P))
            nc.sync.dma_start(out=wo_sb, in_=wo.ap().rearrange("(h p) e -> p h e", p=P))
            dm_sb = consts.tile([P, 4, 512], f32)
            nc.sync.dma_start(out=dm_sb, in_=dm.ap().rearrange("(d p) i -> p d i", p=P))
            ones_col_f = consts.tile([P, 1], f32)
            nc.any.memset(ones_col_f, 1.0)
            ones_row_f = consts.tile([1, P], f32)
            nc.any.memset(ones_row_f, 1.0)
            ones_col = consts.tile([P, 1], f32r)   # K=128, M=1 sums weight
            nc.vector.tensor_copy(ones_col, ones_col_f)
            ones_row = consts.tile([1, P], f32r)   # K=1, M=128 broadcast weight
            nc.vector.tensor_copy(ones_row, ones_row_f)

            def body():
                for b in range(B):
                    _batch(b)

            def _batch(b):
                # ---------- QKV projections for batch b ----------
                qt_sb = qkp.tile([P, HPC, S], f32r, tag="qt")
                kt_sb = qkp.tile([P, HPC, S], f32r, tag="kt")
                v_sb = vpool.tile([P, NJT, FPC], f32r, tag="v")
                for g in range(NTG):
                    toff = b * S + g * 512
                    qps = [ps.tile([P, 512], f32, tag="ps", name=f"qp{b}_{g}_{i}")
                           for i in range(HPC)]
                    kps = [ps.tile([P, 512], f32, tag="ps", name=f"kp{b}_{g}_{i}")
                           for i in range(HPC)]
                    vps = [ps.tile([P, FPC], f32, tag="ps", name=f"vp{b}_{g}_{i}")
                           for i in range(4)]
                    for kc in range(KC):
                        xt_t = xp.tile([P, 512], f32r, tag="xt")
                        nc.sync.dma_start(
                            out=xt_t, in_=xt.ap()[kc * P:(kc + 1) * P, toff:toff + 512]
                        )
                        st = dict(start=(kc == 0), stop=(kc == KC - 1))
                        for m in range(HPC):
                            nc.tensor.matmul(
                                qps[m], wq_sb[:, kc, m * P:(m + 1) * P],
                                xt_t, **st
                            )
                            nc.tensor.matmul(
                                kps[m], wk_sb[:, kc, m * P:(m + 1) * P],
                                xt_t, **st
                            )
                        for ts in range(4):
                            nc.tensor.matmul(
                                vps[ts], xt_t[:, ts * P:(ts + 1) * P],
                                wv_sb[:, kc, :], **st
                            )
                    for m in range(HPC):
                        nc.scalar.copy(
                            qt_sb[:, m, g * 512:(g + 1) * 512], qps[m]
                        )
                        nc.vector.tensor_copy(
                            kt_sb[:, m, g * 512:(g + 1) * 512], kps[m]
                        )
                    for ts in range(4):
                        eng = nc.scalar.copy if ts % 2 else nc.vector.tensor_copy
                        eng(v_sb[:, g * 4 + ts, :], vps[ts])

                # ---------- attention per head ----------
                ot_sb = otp_pool.tile([P, HPC, S], f32r, tag="ot")
                for h in range(HPC):
                    for it in range(NIT):
                        otp = ps.tile([P, 512], f32, tag="ps")
                        sp = ps.tile([1, 512], f32, tag="ps")
                        jts = range(4 * it + 4) if causal else range(NJT)
                        jlast = (4 * it + 3) if causal else (NJT - 1)
                        for jt in jts:
                            lp = ps.tile([P, 512], f32, tag="ps")
                            nc.tensor.matmul(
                                lp, kt_sb[:, h, jt * P:(jt + 1) * P],
                                qt_sb[:, h, it * 512:(it + 1) * 512],
                                start=True, stop=True,
                            )
                            e = ep.tile([P, 512], f32r, tag="e")
                            nc.scalar.activation(
                                e, lp, mybir.ActivationFunctionType.Exp,
                                scale=SCALE,
                            )
                            if causal and jt >= 4 * it:
                                nc.vector.tensor_mul(e, e, dm_sb[:, jt - 4 * it, :])
                            stj = dict(start=(jt == 0), stop=(jt == jlast))
                            nc.tensor.matmul(
                                otp, v_sb[:, jt, h * P:(h + 1) * P],
                                e, **stj
                            )
                            nc.tensor.matmul(sp, ones_col, e, **stj)
                        rinv = smp.tile([1, 512], f32r, tag="rinv")
                        nc.vector.reciprocal(rinv, sp)
                        rbc = ps.tile([P, 512], f32, tag="ps")
                        nc.tensor.matmul(rbc, ones_row, rinv,
                                         start=True, stop=True)
                        rbs = smp.tile([P, 512], f32, tag="rbs")
                        nc.vector.tensor_copy(rbs, rbc)
                        nc.vector.tensor_mul(
                            ot_sb[:, h, it * 512:(it + 1) * 512], otp, rbs
                        )

                # ---------- partial output projection for batch b ----------
                for mt in range(S // P):
                    for nt in range(D // 512):
                        op = ps.tile([P, 512], f32, tag="ps")
                        for h in range(HPC):
                            nc.tensor.matmul(
                                op, ot_sb[:, h, mt * P:(mt + 1) * P],
                                wo_sb[:, h, nt * 512:(nt + 1) * 512],
                                start=(h == 0), stop=(h == HPC - 1),
                            )
                        ob = obp.tile([P, 512], f32, tag="ob")
                        if (mt + nt) % 2:
                            nc.scalar.copy(ob, op)
                        else:
                            nc.vector.tensor_copy(ob, op)
                        nc.sync.dma_start(
                            out=out.ap()[
                                b * S + mt * P: b * S + (mt + 1) * P,
                                nt * 512:(nt + 1) * 512,
                            ],
                            in_=ob,
                        )

            if iters > 1:
                with tc.For_i(0, iters, 1):
                    body()
            else:
                body()
    nc.compile()
    return nc


def _get_nc(causal: bool):
    if causal not in _COMPILED:
        _COMPILED[causal] = _build_nc(causal)
    return _COMPILED[causal]


def _numpy_reference(x, mask, Wq, Aq, Bq, Wk, Ak, Bk, Wv, Av, Bv, Wo, Ao, Bo):
    def lora(x2, W, A, Bm):
        return x2 @ W.T + (x2 @ A.T) @ Bm.T

    b, s, d = x.shape
    x2 = x.reshape(b * s, d)

    def heads(t):
        return t.reshape(b, s, H, HD).transpose(0, 2, 1, 3)

    Q = heads(lora(x2, Wq, Aq, Bq).reshape(b, s, d))
    K = heads(lora(x2, Wk, Ak, Bk).reshape(b, s, d))
    V = heads(lora(x2, Wv, Av, Bv).reshape(b, s, d))
    attn = np.einsum("bhqd,bhkd->bhqk", Q, K) / math.sqrt(HD)
    attn = np.where(mask == 0, np.float32(-1e9), attn)
    attn = attn - attn.max(axis=-1, keepdims=True)
    attn = np.exp(attn)
    attn = attn / attn.sum(axis=-1, keepdims=True)
    o = np.einsum("bhqk,bhkd->bhqd", attn, V)
    o = o.transpose(0, 2, 1, 3).reshape(b * s, d)
    return lora(o, Wo, Ao, Bo).reshape(b, s, d).astype(np.float32)


def kernel(x, mask, Wq, Aq, Bq, Wk, Ak, Bk, Wv, Av, Bv, Wo, Ao, Bo):
    from concourse.bass_utils import run_bass_kernel_spmd

    x = np.asarray(x, dtype=np.float32)
    m2 = np.asarray(mask).reshape(S, S)
    if np.array_equal(m2, np.tril(np.ones((S, S), m2.dtype))):
        causal = True
    elif np.all(m2 != 0):
        causal = False
    else:
        return _numpy_reference(
            np.asarray(x), np.asarray(mask),
            *(np.asarray(a) for a in (Wq, Aq, Bq, Wk, Ak, Bk, Wv, Av, Bv, Wo, Ao, Bo)),
        )

    # Fold LoRA into effective (transposed) weights: W_eff^T = W^T + A^T B^T.
    wqt = (np.asarray(Wq).T + np.asarray(Aq).T @ np.asarray(Bq).T).astype(np.float32)
    wkt = (np.asarray(Wk).T + np.asarray(Ak).T @ np.asarray(Bk).T).astype(np.float32)
    wvt = (np.asarray(Wv).T + np.asarray(Av).T @ np.asarray(Bv).T).astype(np.float32)
    wot = (np.asarray(Wo).T + np.asarray(Ao).T @ np.asarray(Bo).T).astype(np.float32)

    xt = np.ascontiguousarray(x.reshape(T, D).T)

    # Diagonal-crossing causal masks: tile (jt=4*it+d, it): keep where
    # 128*d + p_j <= f_i.
    dmn = np.zeros((4, P, 512), dtype=np.float32)
    for dd in range(4):
        pj = np.arange(P)[:, None]
        fi = np.arange(512)[None, :]
        dmn[dd] = (P * dd + pj <= fi).astype(np.float32)
    dmn = dmn.reshape(4 * P, 512)

    nc = _get_nc(causal)
    in_maps = []
    for c in range(NCORES):
        fs = slice(c * FPC, (c + 1) * FPC)
        in_maps.append({
            "xt": xt,
            "wq": np.ascontiguousarray(wqt[:, fs]),
            "wk": np.ascontiguousarray(wkt[:, fs]),
            "wv": np.ascontiguousarray(wvt[:, fs]),
            "wo": np.ascontiguousarray(wot[fs, :]),
            "dm": dmn,
        })
    res = run_bass_kernel_spmd(
        nc, in_maps, list(range(NCORES)),
        trace=bool(int(os.environ.get("KERNEL_TRACE", "0"))),
    )
    if os.environ.get("KERNEL_TRACE") and res.exec_time_ns is not None:
        print(f"HW exec time: {res.exec_time_ns} ns", file=sys.stderr)
    total = np.zeros((T, D), dtype=np.float32)
    for r in res.results:
        total += r["out"]
    return total.reshape(B, S, D)


# revision 14
# speedup vs baseline: 3.4540x; 1.2130x over previous
"""LoRA-attention TRN2 kernel: head-tensor-parallel over 8 NeuronCores.

Problem (hardcoded): x [4, 2048, 2048] f32, causal mask, H=16 heads, HD=128,
LoRA rank 16 on all four projections.

Strategy:
  - Host folds LoRA into the weights:  W_eff^T = W^T + A^T @ B^T  (exact).
  - Tensor-parallel over heads: core c owns heads {2c, 2c+1} = feature slice
    [c*256, (c+1)*256).  Each core computes Q^T/K^T (feature-major) and V
    (token-major) for its slice, causal flash-style attention per (batch,
    head) with logits kept transposed ([key, query]) so the attention
    contraction stays on the partition dim, then a partial output projection
    over its 256 features.  Host sums the 8 partial outputs.
  - Softmax: no max-subtraction (logits are O(1); exp of masked -inf never
    occurs because masked tiles are either skipped or zeroed post-exp).
    Column sums via ones-vector matmul on the PE; normalization applied to
    O^T at the end via a PE broadcast (ones outer product) + DVE multiply.
"""

import math
import os
import sys

import numpy as np

sys.path.insert(0, "/opt/trn_rl_repo")

B, S, D, H, R = 4, 2048, 2048, 16, 16
HD = D // H              # 128
NCORES = 8
HPC = H // NCORES        # heads per core = 2
FPC = HPC * HD           # features per core = 256
T = B * S                # 8192 tokens
P = 128
SCALE = 1.0 / math.sqrt(HD)

_COMPILED = {}


def _build_nc(causal: bool, iters: int = 1):
    import concourse.mybir as mybir
    import concourse.tile as tile
    from concourse import bacc

    f32 = mybir.dt.float32
    f32r = mybir.dt.float32r
    nc = bacc.Bacc("TRN2", target_bir_lowering=False, debug=False)

    xt = nc.dram_tensor("xt", [D, T], f32r, kind="ExternalInput")
    wq = nc.dram_tensor("wq", [D, FPC], f32r, kind="ExternalInput")
    wk = nc.dram_tensor("wk", [D, FPC], f32r, kind="ExternalInput")
    wv = nc.dram_tensor("wv", [D, FPC], f32r, kind="ExternalInput")
    wo = nc.dram_tensor("wo", [FPC, D], f32r, kind="ExternalInput")
    dm = nc.dram_tensor("dm", [4 * P, 512], f32, kind="ExternalInput")
    out = nc.dram_tensor("out", [T, D], f32, kind="ExternalOutput")

    KC = D // P            # 16 contraction chunks for projections
    NTG = S // 512         # 4 token groups per batch
    NIT = S // 512         # 4 query tiles per (b, h)
    NJT = S // P           # 16 key chunks per (b, h)

    with tile.TileContext(nc) as tc:
        with (
            nc.allow_low_precision(reason="f32r matmul pipeline; fp32 PSUM accum"),
            tc.tile_pool(name="consts", bufs=1) as consts,
            tc.tile_pool(name="xp", bufs=4) as xp,
            tc.tile_pool(name="qk", bufs=1) as qkp,
            tc.tile_pool(name="vp", bufs=1) as vpool,
            tc.tile_pool(name="ep", bufs=4) as ep,
            tc.tile_pool(name="ot", bufs=1) as otp_pool,
            tc.tile_pool(name="sm", bufs=4) as smp,
            tc.tile_pool(name="ob", bufs=4) as obp,
            tc.tile_pool(name="ps", bufs=8, space="PSUM") as ps,
        ):
            # ---- resident constants ----
            wq_sb = consts.tile([P, KC, FPC], f32r)
            wk_sb = consts.tile([P, KC, FPC], f32r)
            wv_sb = consts.tile([P, KC, FPC], f32r)
            wo_sb = consts.tile([P, HPC, D], f32r)
            nc.sync.dma_start(out=wq_sb, in_=wq.ap().rearrange("(c p) f -> p c f", p=P))
            nc.sync.dma_start(out=wk_sb, in_=wk.ap().rearrange("(c p) f -> p c f", p=P))
            nc.sync.dma_start(out=wv_sb, in_=wv.ap().rearrange("(c p) f -> p c f", p=P))
            nc.sync.dma_start(out=wo_sb, in_=wo.ap().rearrange("(h p) e -> p h e", p=P))
            dm_sb = consts.tile([P, 4, 512], f32)
            nc.sync.dma_start(out=dm_sb, in_=dm.ap().rearrange("(d p) i -> p d i", p=P))
            ones_col_f = consts.tile([P, 1], f32)
            nc.any.memset(ones_col_f, 1.0)
            ones_row_f = consts.tile([1, P], f32)
            nc.any.memset(ones_row_f, 1.0)
            ones_col = consts.tile([P, 1], f32r)   # K=128, M=1 sums weight
            nc.vector.tensor_copy(ones_col, ones_col_f)
            ones_row = consts.tile([1, P], f32r)   # K=1, M=128 broadcast weight
            nc.vector.tensor_copy(ones_row, ones_row_f)

            def body():
                for b in range(B):
                    _batch(b)

            def _batch(b):
                # ---------- QKV projections for batch b ----------
                qt_sb = qkp.tile([P, HPC, S], f32r, tag="qt")
                kt_sb = qkp.tile([P, HPC, S], f32r, tag="kt")
                v_sb = vpool.tile([P, NJT, FPC], f32r, tag="v")
                for g in range(NTG):
                    toff = b * S + g * 512
                    qps = [ps.tile([P, 512], f32, tag="ps", name=f"qp{b}_{g}_{i}")
                           for i in range(HPC)]
                    kps = [ps.tile([P, 512], f32, tag="ps", name=f"kp{b}_{g}_{i}")
                           for i in range(HPC)]
                    vps = [ps.tile([P, FPC], f32, tag="ps", name=f"vp{b}_{g}_{i}")
                           for i in range(4)]
                    for kc in range(KC):
                        xt_t = xp.tile([P, 512], f32r, tag="xt")
                        nc.sync.dma_start(
                            out=xt_t, in_=xt.ap()[kc * P:(kc + 1) * P, toff:toff + 512]
                        )
                        st = dict(start=(kc == 0), stop=(kc == KC - 1))
                        for m in range(HPC):
                            nc.tensor.matmul(
                                qps[m], wq_sb[:, kc, m * P:(m + 1) * P],
                                xt_t, **st
                            )
                            nc.tensor.matmul(
                                kps[m], wk_sb[:, kc, m * P:(m + 1) * P],
                                xt_t, **st
                            )
                        for ts in range(4):
                            nc.tensor.matmul(
                                vps[ts], xt_t[:, ts * P:(ts + 1) * P],
                                wv_sb[:, kc, :], **st
                            )
                    for m in range(HPC):
                        nc.scalar.copy(
                            qt_sb[:, m, g * 512:(g + 1) * 512], qps[m]
                        )
                        nc.vector.tensor_copy(
                            kt_sb[:, m, g * 512:(g + 1) * 512], kps[m]
                        )
                    for ts in range(4):
                        eng = nc.scalar.copy if ts % 2 else nc.vector.tensor_copy
                        eng(v_sb[:, g * 4 + ts, :], vps[ts])

                # ---------- attention per head (jt-outer: K/V/ones weight
                # loads amortized across query tiles) ----------
                ot_sb = otp_pool.tile([P, HPC, S], f32r, tag="ot")
                for h in range(HPC):
                    for ip in range(NIT // 2):
                        pits = (2 * ip, 2 * ip + 1)
                        otps = {it: ps.tile([P, 512], f32, tag="ps",
                                            name=f"otp{b}_{h}_{it}")
                                for it in pits}
                        sps = {it: ps.tile([1, 512], f32, tag="ps",
                                           name=f"sp{b}_{h}_{it}")
                               for it in pits}
                        jmax = (4 * pits[-1] + 4) if causal else NJT
                        for jt in range(jmax):
                            its = [it for it in pits
                                   if (not causal) or jt <= 4 * it + 3]
                            es = {}
                            for it in its:
                                lp = ps.tile([P, 512], f32, tag="ps")
                                nc.tensor.matmul(
                                    lp, kt_sb[:, h, jt * P:(jt + 1) * P],
                                    qt_sb[:, h, it * 512:(it + 1) * 512],
                                    start=True, stop=True,
                                )
                                e = ep.tile([P, 512], f32r, tag="e")
                                nc.scalar.activation(
                                    e, lp, mybir.ActivationFunctionType.Exp,
                                    scale=SCALE,
                                )
                                if causal and it == jt // 4:
                                    nc.vector.tensor_mul(
                                        e, e, dm_sb[:, jt - 4 * it, :])
                                es[it] = e
                            for it in its:
                                stj = dict(
                                    start=(jt == 0),
                                    stop=(jt == ((4 * it + 3) if causal
                                                 else NJT - 1)),
                                )
                                nc.tensor.matmul(
                                    otps[it], v_sb[:, jt, h * P:(h + 1) * P],
                                    es[it], **stj
                                )
                            for it in its:
                                stj = dict(
                                    start=(jt == 0),
                                    stop=(jt == ((4 * it + 3) if causal
                                                 else NJT - 1)),
                                )
                                nc.tensor.matmul(sps[it], ones_col,
                                                 es[it], **stj)
                        for it in pits:
                            rinv = smp.tile([1, 512], f32r, tag="rinv")
                            nc.vector.reciprocal(rinv, sps[it])
                            rbc = ps.tile([P, 512], f32, tag="ps")
                            nc.tensor.matmul(rbc, ones_row, rinv,
                                             start=True, stop=True)
                            rbs = smp.tile([P, 512], f32, tag="rbs")
                            nc.vector.tensor_copy(rbs, rbc)
                            nc.vector.tensor_mul(
                                ot_sb[:, h, it * 512:(it + 1) * 512],
                                otps[it], rbs
                            )

                # ---------- partial output projection for batch b ----------
                # mt-outer with h inner: O^T weight loads reused across nt.
                for mt in range(S // P):
                    ops = [ps.tile([P, 512], f32, tag="ps", name=f"op{b}_{mt}_{i}")
                           for i in range(4)]
                    for h in range(HPC):
                        for nt in range(4):
                            nc.tensor.matmul(
                                ops[nt], ot_sb[:, h, mt * P:(mt + 1) * P],
                                wo_sb[:, h, nt * 512:(nt + 1) * 512],
                                start=(h == 0), stop=(h == HPC - 1),
                            )
                    for nt in range(4):
                        ob = obp.tile([P, 512], f32, tag="ob")
                        if (mt + nt) % 2:
                            nc.scalar.copy(ob, ops[nt])
                        else:
                            nc.vector.tensor_copy(ob, ops[nt])
                        nc.sync.dma_start(
                            out=out.ap()[
                                b * S + mt * P: b * S + (mt + 1) * P,
                                nt * 512:(nt + 1) * 512,
                            ],
                            in_=ob,
                        )

            if iters > 1:
                with tc.For_i(0, iters, 1):
                    body()
            else:
                body()
    nc.compile()
    return nc


def _get_nc(causal: bool):
    if causal not in _COMPILED:
        _COMPILED[causal] = _build_nc(causal)
    return _COMPILED[causal]


def _numpy_reference(x, mask, Wq, Aq, Bq, Wk, Ak, Bk, Wv, Av, Bv, Wo, Ao, Bo):
    def lora(x2, W, A, Bm):
        return x2 @ W.T + (x2 @ A.T) @ Bm.T

    b, s, d = x.shape
    x2 = x.reshape(b * s, d)

    def heads(t):
        return t.reshape(b, s, H, HD).transpose(0, 2, 1, 3)

    Q = heads(lora(x2, Wq, Aq, Bq).reshape(b, s, d))
    K = heads(lora(x2, Wk, Ak, Bk).reshape(b, s, d))
    V = heads(lora(x2, Wv, Av, Bv).reshape(b, s, d))
    attn = np.einsum("bhqd,bhkd->bhqk", Q, K) / math.sqrt(HD)
    attn = np.where(mask == 0, np.float32(-1e9), attn)
    attn = attn - attn.max(axis=-1, keepdims=True)
    attn = np.exp(attn)
    attn = attn / attn.sum(axis=-1, keepdims=True)
    o = np.einsum("bhqk,bhkd->bhqd", attn, V)
    o = o.transpose(0, 2, 1, 3).reshape(b * s, d)
    return lora(o, Wo, Ao, Bo).reshape(b, s, d).astype(np.float32)


def kernel(x, mask, Wq, Aq, Bq, Wk, Ak, Bk, Wv, Av, Bv, Wo, Ao, Bo):
    from concourse.bass_utils import run_bass_kernel_spmd

    x = np.asarray(x, dtype=np.float32)
    m2 = np.asarray(mask).reshape(S, S)
    if np.array_equal(m2, np.tril(np.ones((S, S), m2.dtype))):
        causal = True
    elif np.all(m2 != 0):
        causal = False
    else:
        return _numpy_reference(
            np.asarray(x), np.asarray(mask),
            *(np.asarray(a) for a in (Wq, Aq, Bq, Wk, Ak, Bk, Wv, Av, Bv, Wo, Ao, Bo)),
        )

    # Fold LoRA into effective (transposed) weights: W_eff^T = W^T + A^T B^T.
    wqt = (np.asarray(Wq).T + np.asarray(Aq).T @ np.asarray(Bq).T).astype(np.float32)
    wkt = (np.asarray(Wk).T + np.asarray(Ak).T @ np.asarray(Bk).T).astype(np.float32)
    wvt = (np.asarray(Wv).T + np.asarray(Av).T @ np.asarray(Bv).T).astype(np.float32)
    wot = (np.asarray(Wo).T + np.asarray(Ao).T @ np.asarray(Bo).T).astype(np.float32)

    xt = np.ascontiguousarray(x.reshape(T, D).T)

    # Diagonal-crossing causal masks: tile (jt=4*it+d, it): keep where
    # 128*d + p_j <= f_i.
    dmn = np.zeros((4, P, 512), dtype=np.float32)
    for dd in range(4):
        pj = np.arange(P)[:, None]
        fi = np.arange(512)[None, :]
        dmn[dd] = (P * dd + pj <= fi).astype(np.float32)
    dmn = dmn.reshape(4 * P, 512)

    nc = _get_nc(causal)
    in_maps = []
    for c in range(NCORES):
        fs = slice(c * FPC, (c + 1) * FPC)
        in_maps.append({
            "xt": xt,
            "wq": np.ascontiguousarray(wqt[:, fs]),
            "wk": np.ascontiguousarray(wkt[:, fs]),
            "wv": np.ascontiguousarray(wvt[:, fs]),
            "wo": np.ascontiguousarray(wot[fs, :]),
            "dm": dmn,
        })
    res = run_bass_kernel_spmd(
        nc, in_maps, list(range(NCORES)),
        trace=bool(int(os.environ.get("KERNEL_TRACE", "0"))),
    )
    if os.environ.get("KERNEL_TRACE") and res.exec_time_ns is not None:
        print(f"HW exec time: {res.exec_time_ns} ns", file=sys.stderr)
    total = np.zeros((T, D), dtype=np.float32)
    for r in res.results:
        total += r["out"]
    return total.reshape(B, S, D)


# revision 16
# speedup vs baseline: 3.9063x; 1.1310x over previous
"""LoRA-attention TRN2 kernel: head-tensor-parallel over 8 NeuronCores.

Problem (hardcoded): x [4, 2048, 2048] f32, causal mask, H=16 heads, HD=128,
LoRA rank 16 on all four projections.

Strategy:
  - Host folds LoRA into the weights:  W_eff^T = W^T + A^T @ B^T  (exact).
  - Tensor-parallel over heads: core c owns heads {2c, 2c+1} = feature slice
    [c*256, (c+1)*256).  Each core computes Q^T/K^T (feature-major) and V
    (token-major) for its slice, causal flash-style attention per (batch,
    head) with logits kept transposed ([key, query]) so the attention
    contraction stays on the partition dim, then a partial output projection
    over its 256 features.  Host sums the 8 partial outputs.
  - Softmax: no max-subtraction (logits are O(1); exp of masked -inf never
    occurs because masked tiles are either skipped or zeroed post-exp).
    Column sums via ones-vector matmul on the PE; normalization applied to
    O^T at the end via a PE broadcast (ones outer product) + DVE multiply.
"""

import math
import os
import sys

import numpy as np

sys.path.insert(0, "/opt/trn_rl_repo")

B, S, D, H, R = 4, 2048, 2048, 16, 16
HD = D // H              # 128
NCORES = 8
HPC = H // NCORES        # heads per core = 2
FPC = HPC * HD           # features per core = 256
T = B * S                # 8192 tokens
P = 128
SCALE = 1.0 / math.sqrt(HD)

_COMPILED = {}


def _build_nc(causal: bool, iters: int = 1):
    import concourse.mybir as mybir
    import concourse.tile as tile
    from concourse import bacc

    f32 = mybir.dt.float32
    f32r = mybir.dt.float32r
    nc = bacc.Bacc("TRN2", target_bir_lowering=False, debug=False)

    xt = nc.dram_tensor("xt", [D, T], f32r, kind="ExternalInput")
    wq = nc.dram_tensor("wq", [D, FPC], f32r, kind="ExternalInput")
    wk = nc.dram_tensor("wk", [D, FPC], f32r, kind="ExternalInput")
    wv = nc.dram_tensor("wv", [D, FPC], f32r, kind="ExternalInput")
    wo = nc.dram_tensor("wo", [FPC, D], f32r, kind="ExternalInput")
    dm = nc.dram_tensor("dm", [4 * P, 512], f32, kind="ExternalInput")
    out = nc.dram_tensor("out", [T, D], f32, kind="ExternalOutput")

    KC = D // P            # 16 contraction chunks for projections
    NTG = S // 512         # 4 token groups per batch
    NIT = S // 512         # 4 query tiles per (b, h)
    NJT = S // P           # 16 key chunks per (b, h)

    with tile.TileContext(nc) as tc:
        with (
            nc.allow_low_precision(reason="f32r matmul pipeline; fp32 PSUM accum"),
            tc.tile_pool(name="consts", bufs=1) as consts,
            tc.tile_pool(name="xp", bufs=4) as xp,
            tc.tile_pool(name="qk", bufs=1) as qkp,
            tc.tile_pool(name="vp", bufs=1) as vpool,
            tc.tile_pool(name="ep", bufs=4) as ep,
            tc.tile_pool(name="ot", bufs=1) as otp_pool,
            tc.tile_pool(name="sm", bufs=4) as smp,
            tc.tile_pool(name="ob", bufs=3) as obp,
            tc.tile_pool(name="ps", bufs=8, space="PSUM") as ps,
        ):
            # ---- resident constants ----
            wq_sb = consts.tile([P, KC, FPC], f32r)
            wk_sb = consts.tile([P, KC, FPC], f32r)
            wv_sb = consts.tile([P, KC, FPC], f32r)
            wo_sb = consts.tile([P, HPC, D], f32r)
            nc.sync.dma_start(out=wq_sb, in_=wq.ap().rearrange("(c p) f -> p c f", p=P))
            nc.sync.dma_start(out=wk_sb, in_=wk.ap().rearrange("(c p) f -> p c f", p=P))
            nc.sync.dma_start(out=wv_sb, in_=wv.ap().rearrange("(c p) f -> p c f", p=P))
            nc.sync.dma_start(out=wo_sb, in_=wo.ap().rearrange("(h p) e -> p h e", p=P))
            dm_sb = consts.tile([P, 4, 512], f32)
            nc.sync.dma_start(out=dm_sb, in_=dm.ap().rearrange("(d p) i -> p d i", p=P))
            ones_col_f = consts.tile([P, 1], f32)
            nc.any.memset(ones_col_f, 1.0)
            ones_row_f = consts.tile([1, P], f32)
            nc.any.memset(ones_row_f, 1.0)
            ones_col = consts.tile([P, 1], f32r)   # K=128, M=1 sums weight
            nc.vector.tensor_copy(ones_col, ones_col_f)
            ones_row = consts.tile([1, P], f32r)   # K=1, M=128 broadcast weight
            nc.vector.tensor_copy(ones_row, ones_row_f)

            def body():
                for b in range(B):
                    _batch(b)

            def _batch(b):
                # ---------- QKV projections for batch b ----------
                qt_sb = qkp.tile([P, HPC, S], f32r, tag="qt")
                kt_sb = qkp.tile([P, HPC, S], f32r, tag="kt")
                v_sb = vpool.tile([P, NJT, FPC], f32r, tag="v")
                for g in range(NTG):
                    toff = b * S + g * 512
                    qps = [ps.tile([P, 512], f32, tag="ps", name=f"qp{b}_{g}_{i}")
                           for i in range(HPC)]
                    kps = [ps.tile([P, 512], f32, tag="ps", name=f"kp{b}_{g}_{i}")
                           for i in range(HPC)]
                    vps = [ps.tile([P, FPC], f32, tag="ps", name=f"vp{b}_{g}_{i}")
                           for i in range(4)]
                    for kc2 in range(KC // 2):
                        xt_t = xp.tile([P, 2, 512], f32r, tag="xt")
                        nc.sync.dma_start(
                            out=xt_t,
                            in_=xt.ap()[
                                kc2 * 2 * P:(kc2 + 1) * 2 * P, toff:toff + 512
                            ].rearrange("(two p) t -> p two t", p=P),
                        )
                        for j in range(2):
                            kc = 2 * kc2 + j
                            st = dict(start=(kc == 0), stop=(kc == KC - 1))
                            for m in range(HPC):
                                nc.tensor.matmul(
                                    qps[m], wq_sb[:, kc, m * P:(m + 1) * P],
                                    xt_t[:, j, :], **st
                                )
                                nc.tensor.matmul(
                                    kps[m], wk_sb[:, kc, m * P:(m + 1) * P],
                                    xt_t[:, j, :], **st
                                )
                            for ts in range(4):
                                nc.tensor.matmul(
                                    vps[ts], xt_t[:, j, ts * P:(ts + 1) * P],
                                    wv_sb[:, kc, :], **st
                                )
                    for m in range(HPC):
                        nc.scalar.copy(
                            qt_sb[:, m, g * 512:(g + 1) * 512], qps[m]
                        )
                        nc.vector.tensor_copy(
                            kt_sb[:, m, g * 512:(g + 1) * 512], kps[m]
                        )
                    for ts in range(4):
                        eng = nc.scalar.copy if ts % 2 else nc.vector.tensor_copy
                        eng(v_sb[:, g * 4 + ts, :], vps[ts])

                # ---------- attention per head (jt-outer: K/V/ones weight
                # loads amortized across query tiles) ----------
                ot_sb = otp_pool.tile([P, HPC, S], f32r, tag="ot")
                for h in range(HPC):
                    for ip in range(NIT // 2):
                        pits = (2 * ip, 2 * ip + 1)
                        otps = {it: ps.tile([P, 512], f32, tag="ps",
                                            name=f"otp{b}_{h}_{it}")
                                for it in pits}
                        sps = {it: ps.tile([1, 512], f32, tag="ps",
                                           name=f"sp{b}_{h}_{it}")
                               for it in pits}
                        jmax = (4 * pits[-1] + 4) if causal else NJT
                        for jt in range(jmax):
                            its = [it for it in pits
                                   if (not causal) or jt <= 4 * it + 3]
                            es = {}
                            for it in its:
                                lp = ps.tile([P, 512], f32, tag="ps")
                                nc.tensor.matmul(
                                    lp, kt_sb[:, h, jt * P:(jt + 1) * P],
                                    qt_sb[:, h, it * 512:(it + 1) * 512],
                                    start=True, stop=True,
                                )
                                e = ep.tile([P, 512], f32r, tag="e")
                                nc.scalar.activation(
                                    e, lp, mybir.ActivationFunctionType.Exp,
                                    scale=SCALE,
                                )
                                if causal and it == jt // 4:
                                    nc.vector.tensor_mul(
                                        e, e, dm_sb[:, jt - 4 * it, :])
                                es[it] = e
                            for it in its:
                                stj = dict(
                                    start=(jt == 0),
                                    stop=(jt == ((4 * it + 3) if causal
                                                 else NJT - 1)),
                                )
                                nc.tensor.matmul(
                                    otps[it], v_sb[:, jt, h * P:(h + 1) * P],
                                    es[it], **stj
                                )
                            for it in its:
                                stj = dict(
                                    start=(jt == 0),
                                    stop=(jt == ((4 * it + 3) if causal
                                                 else NJT - 1)),
                                )
                                nc.tensor.matmul(sps[it], ones_col,
                                                 es[it], **stj)
                        for it in pits:
                            rinv = smp.tile([1, 512], f32, tag="rinv")
                            nc.vector.reciprocal(rinv, sps[it])
                            rbs = smp.tile([P, 512], f32, tag="rbs")
                            nc.gpsimd.partition_broadcast(rbs, rinv)
                            nc.vector.tensor_mul(
                                ot_sb[:, h, it * 512:(it + 1) * 512],
                                otps[it], rbs
                            )

                # ---------- partial output projection for batch b ----------
                # mt-outer with h inner: O^T weight loads reused across nt.
                for mt in range(S // P):
                    ops = [ps.tile([P, 512], f32, tag="ps", name=f"op{b}_{mt}_{i}")
                           for i in range(4)]
                    for h in range(HPC):
                        for nt in range(4):
                            nc.tensor.matmul(
                                ops[nt], ot_sb[:, h, mt * P:(mt + 1) * P],
                                wo_sb[:, h, nt * 512:(nt + 1) * 512],
                                start=(h == 0), stop=(h == HPC - 1),
                            )
                    ob4 = obp.tile([P, 2048], f32, tag="ob")
                    for nt in range(4):
                        if (mt + nt) % 2:
                            nc.scalar.copy(ob4[:, nt * 512:(nt + 1) * 512],
                                           ops[nt])
                        else:
                            nc.vector.tensor_copy(
                                ob4[:, nt * 512:(nt + 1) * 512], ops[nt])
                    nc.sync.dma_start(
                        out=out.ap()[b * S + mt * P: b * S + (mt + 1) * P, :],
                        in_=ob4,
                    )

            if iters > 1:
                with tc.For_i(0, iters, 1):
                    body()
            else:
                body()
    nc.compile()
    return nc


def _get_nc(causal: bool):
    if causal not in _COMPILED:
        _COMPILED[causal] = _build_nc(causal)
    return _COMPILED[causal]


def _numpy_reference(x, mask, Wq, Aq, Bq, Wk, Ak, Bk, Wv, Av, Bv, Wo, Ao, Bo):
    def lora(x2, W, A, Bm):
        return x2 @ W.T + (x2 @ A.T) @ Bm.T

    b, s, d = x.shape
    x2 = x.reshape(b * s, d)

    def heads(t):
        return t.reshape(b, s, H, HD).transpose(0, 2, 1, 3)

    Q = heads(lora(x2, Wq, Aq, Bq).reshape(b, s, d))
    K = heads(lora(x2, Wk, Ak, Bk).reshape(b, s, d))
    V = heads(lora(x2, Wv, Av, Bv).reshape(b, s, d))
    attn = np.einsum("bhqd,bhkd->bhqk", Q, K) / math.sqrt(HD)
    attn = np.where(mask == 0, np.float32(-1e9), attn)
    attn = attn - attn.max(axis=-1, keepdims=True)
    attn = np.exp(attn)
    attn = attn / attn.sum(axis=-1, keepdims=True)
    o = np.einsum("bhqk,bhkd->bhqd", attn, V)
    o = o.transpose(0, 2, 1, 3).reshape(b * s, d)
    return lora(o, Wo, Ao, Bo).reshape(b, s, d).astype(np.float32)


def kernel(x, mask, Wq, Aq, Bq, Wk, Ak, Bk, Wv, Av, Bv, Wo, Ao, Bo):
    from concourse.bass_utils import run_bass_kernel_spmd

    x = np.asarray(x, dtype=np.float32)
    m2 = np.asarray(mask).reshape(S, S)
    if np.array_equal(m2, np.tril(np.ones((S, S), m2.dtype))):
        causal = True
    elif np.all(m2 != 0):
        causal = False
    else:
        return _numpy_reference(
            np.asarray(x), np.asarray(mask),
            *(np.asarray(a) for a in (Wq, Aq, Bq, Wk, Ak, Bk, Wv, Av, Bv, Wo, Ao, Bo)),
        )

    # Fold LoRA into effective (transposed) weights: W_eff^T = W^T + A^T B^T.
    wqt = (np.asarray(Wq).T + np.asarray(Aq).T @ np.asarray(Bq).T).astype(np.float32)
    wkt = (np.asarray(Wk).T + np.asarray(Ak).T @ np.asarray(Bk).T).astype(np.float32)
    wvt = (np.asarray(Wv).T + np.asarray(Av).T @ np.asarray(Bv).T).astype(np.float32)
    wot = (np.asarray(Wo).T + np.asarray(Ao).T @ np.asarray(Bo).T).astype(np.float32)

    xt = np.ascontiguousarray(x.reshape(T, D).T)

    # Diagonal-crossing causal masks: tile (jt=4*it+d, it): keep where
    # 128*d + p_j <= f_i.
    dmn = np.zeros((4, P, 512), dtype=np.float32)
    for dd in range(4):
        pj = np.arange(P)[:, None]
        fi = np.arange(512)[None, :]
        dmn[dd] = (P * dd + pj <= fi).astype(np.float32)
    dmn = dmn.reshape(4 * P, 512)

    nc = _get_nc(causal)
    in_maps = []
    for c in range(NCORES):
        fs = slice(c * FPC, (c + 1) * FPC)
        in_maps.append({
            "xt": xt,
            "wq": np.ascontiguousarray(wqt[:, fs]),
            "wk": np.ascontiguousarray(wkt[:, fs]),
            "wv": np.ascontiguousarray(wvt[:, fs]),
            "wo": np.ascontiguousarray(wot[fs, :]),
            "dm": dmn,
        })
    res = run_bass_kernel_spmd(
        nc, in_maps, list(range(NCORES)),
        trace=bool(int(os.environ.get("KERNEL_TRACE", "0"))),
    )
    if os.environ.get("KERNEL_TRACE") and res.exec_time_ns is not None:
        print(f"HW exec time: {res.exec_time_ns} ns", file=sys.stderr)
    total = np.zeros((T, D), dtype=np.float32)
    for r in res.results:
        total += r["out"]
    return total.reshape(B, S, D)
